# revision 17
# baseline (speedup 1.0000x reference)
"""Bass/Tile kernel for nn_Attention_81690277970645 on TRN2 (v2).

Sharding: 8 cores = 2 batches x 4 head-groups (4 heads of d=64 each).
Per core (batch bi, head-group hg):
  inputs:  xbf [2048, 1024] bf16, wq/wk/wv slices [1024, 256] bf16,
           bq/bk [256] f32, wo slice [256, 1024] bf16, ones64 [1,64] bf16
  output:  partial out [2048, 1024] f32 (host sums the 4 head-group
           partials per batch and adds bo_eff = bo + bv @ wo)

v2 changes vs v1 (271us):
  1. x^T via DMA-transpose (XBAR) in bf16 -- frees PE transposes and the
     psum->sbuf copy engines entirely.
  2. All matmuls bf16 (weights host-cast); V bias folded into host bo_eff
     (softmax rows sum to 1 => attn @ (1 bv^T) Wo = bv Wo = const row).
  3. Softmax exp split across ACT and DVE per-jt: ACT runs exact Exp
     activation; DVE computes Schraudolph fast-exp in ONE tensor_scalar:
     bitcast_bf16(int16(S * SCALE*128/ln2 + (127*128-7.25))) -- measured
     end-to-end rel err ~5e-3 (gate 2e-2).
  4. Output projection psum -> DRAM directly via DMA (no engine copy).
  5. GPSIMD unusable (cannot access PSUM on TRN2).

Known HW pitfalls: DVE reciprocal() on a 1-partition row costs ~3.3us;
reciprocal_approx_fast returns garbage at base partition != 0; gpsimd
cannot touch PSUM; dma_start_transpose needs contiguous SBUF dest.
"""
import sys
import numpy as np

if '/opt/trn_rl_repo' not in sys.path:
    sys.path.insert(0, '/opt/trn_rl_repo')

import concourse.mybir as mybir
from concourse import bacc
from concourse.tile import TileContext

F32 = mybir.dt.float32
F32R = mybir.dt.float32r
BF16 = mybir.dt.bfloat16
I16 = mybir.dt.int16

SEQ = 2048
DIM = 1024
EMB_C = 256          # per-core emb columns (4 heads x 64)
NH = 4               # heads per core
DH = 64
SCALE = DH ** -0.5
P = 128
NSEQT = SEQ // P     # 16 seq tiles
NDIMC = DIM // P     # 8 dim chunks
NEMBC = EMB_C // P   # 2 emb chunks
IBLK = 512
NIBLK = SEQ // IBLK  # 4 i-blocks
NJT = SEQ // P       # 16 j tiles

LN2 = 0.6931471805599453
EXP_A = SCALE * 128.0 / LN2      # schraudolph scale (SCALE folded in)
EXP_B = 127.0 * 128.0 - 7.25     # schraudolph bias

# which jt's exp runs on DVE (rest on ACT); ~44% DVE
DVE_JT = frozenset((1, 3, 5, 7, 9, 11, 13))


def build_kernel(row_pack=False):
    nc = bacc.Bacc("TRN2", target_bir_lowering=False, debug=False, num_devices=8)

    x = nc.dram_tensor("xbf", [SEQ, DIM], BF16, kind="ExternalInput")
    wq = nc.dram_tensor("wq", [DIM, EMB_C], BF16, kind="ExternalInput")
    wk = nc.dram_tensor("wk", [DIM, EMB_C], BF16, kind="ExternalInput")
    wv = nc.dram_tensor("wv", [DIM, EMB_C], BF16, kind="ExternalInput")
    bq = nc.dram_tensor("bq", [EMB_C], F32, kind="ExternalInput")
    bk = nc.dram_tensor("bk", [EMB_C], F32, kind="ExternalInput")
    wo = nc.dram_tensor("wo", [EMB_C, DIM], BF16, kind="ExternalInput")
    ones_d = nc.dram_tensor("ones64", [1, DH], F32, kind="ExternalInput")
    out = nc.dram_tensor("out", [SEQ, DIM], F32, kind="ExternalOutput")

    with TileContext(nc) as tc:
        with (
            tc.tile_pool(name="xt", bufs=1) as xt_pool,
            tc.tile_pool(name="w", bufs=1) as w_pool,
            tc.tile_pool(name="big", bufs=1) as big_pool,
            tc.tile_pool(name="stage", bufs=3) as stage_pool,
        ):
            # ---- stage A: weights + x^T via DMA transpose ----
            # xT lives in the FIRST pool: the XBAR dma-transpose writes
            # garbage into even partitions when the SBUF dest is not
            # 512B-aligned (a 256B tile allocated before these broke it).
            xT = []
            for b in range(NIBLK):
                t = xt_pool.tile([P, NDIMC, IBLK], BF16, name=f"xT_{b}")
                nc.sync.dma_start_transpose(t[:], x[b * IBLK:(b + 1) * IBLK, :])
                xT.append(t)

            ones_t = w_pool.tile([1, DH], F32R)
            nc.sync.dma_start(ones_t[:], ones_d[:].bitcast(F32R))

            wq_sb = w_pool.tile([P, NDIMC, EMB_C], BF16)
            nc.sync.dma_start(wq_sb[:], wq.rearrange("(c p) e -> p c e", p=P))
            wk_sb = w_pool.tile([P, NDIMC, EMB_C], BF16)
            nc.sync.dma_start(wk_sb[:], wk.rearrange("(c p) e -> p c e", p=P))
            wv_sb = w_pool.tile([P, NDIMC, EMB_C], BF16)
            nc.sync.dma_start(wv_sb[:], wv.rearrange("(c p) e -> p c e", p=P))
            wo_sb = w_pool.tile([P, NEMBC, DIM], BF16)
            nc.sync.dma_start(wo_sb[:], wo.rearrange("(c p) n -> p c n", p=P))
            bq_sb = w_pool.tile([P, NEMBC], F32)
            nc.sync.dma_start(bq_sb[:], bq.rearrange("(c p) -> p c", p=P))
            bk_sb = w_pool.tile([P, NEMBC], F32)
            nc.sync.dma_start(bk_sb[:], bk.rearrange("(c p) -> p c", p=P))

            # ---- stage B: K^T, V, Q^T ----
            psA_ctx = tc.tile_pool(name="psA", bufs=1, space="PSUM")
            psA = psA_ctx.__enter__()

            QT = big_pool.tile([P, NEMBC, SEQ], BF16)
            KT = big_pool.tile([P, NEMBC, SEQ], BF16)
            VP = big_pool.tile([P, NSEQT, NH * (DH + 1)], BF16)
            for h in range(NH):
                nc.vector.memset(VP[:, :, h * (DH + 1) + DH], 1.0)

            def emit_proj(dst, wsb, bsb, e, ib):
                pq = psA.tile([P, IBLK], F32, tag="pq", bufs=2)
                for c in range(NDIMC):
                    nc.tensor.matmul(
                        pq[:],
                        wsb[:, c, e * P:(e + 1) * P],
                        xT[ib][:, c, :],
                        start=(c == 0), stop=(c == NDIMC - 1),
                    )
                nc.scalar.activation(
                    dst[:, e, ib * IBLK:(ib + 1) * IBLK], pq[:],
                    mybir.ActivationFunctionType.Identity,
                    bias=bsb[:, e:e + 1], scale=1.0,
                )

            def emit_v(s):
                ib, si = divmod(s, IBLK // P)
                pv = psA.tile([P, EMB_C], F32, tag="pv", bufs=2)
                for c in range(NDIMC):
                    nc.tensor.matmul(
                        pv[:],
                        xT[ib][:, c, si * P:(si + 1) * P],
                        wv_sb[:, c, :],
                        start=(c == 0), stop=(c == NDIMC - 1),
                    )
                nc.vector.tensor_copy(
                    VP[:, s, :].rearrange("p (h x) -> p h x", h=NH)[:, :, :DH],
                    pv[:].rearrange("p (h d) -> p h d", h=NH),
                )

            for e in range(NEMBC):
                for ib in range(NIBLK):
                    emit_proj(KT, wk_sb, bk_sb, e, ib)
            for s in range(NSEQT):
                emit_v(s)
            for e in range(NEMBC):
                for ib in range(NIBLK):
                    emit_proj(QT, wq_sb, bq_sb, e, ib)

            psA_ctx.__exit__(None, None, None)

            # ---- stages C+D: attention + output projection ----
            psB_ctx = tc.tile_pool(name="psB", bufs=1, space="PSUM")
            psB = psB_ctx.__enter__()
            es_ctx = tc.tile_pool(name="es", bufs=1)
            es_pool = es_ctx.__enter__()

            outT = big_pool.tile([P, NEMBC, SEQ], BF16)

            def emit_spair(ib, jt, hp):
                """S^T for head-pair hp at (ib, jt): 2 matmuls + one exp."""
                i0 = ib * IBLK
                ps = psB.tile([P, 2, IBLK], F32, tag="s0", bufs=2,
                              name=f"ps{hp}_{ib}_{jt}")
                for hh in range(2):
                    lo = hh * DH
                    nc.tensor.matmul(
                        ps[:, hh, :],
                        KT[lo:lo + DH, hp, jt * P:(jt + 1) * P],
                        QT[lo:lo + DH, hp, i0:i0 + IBLK],
                        start=True, stop=True,
                    )
                es = es_pool.tile([P, 2, IBLK], BF16, tag="es", bufs=4,
                                  name=f"es{hp}_{ib}_{jt}")
                if jt in DVE_JT:
                    nc.vector.tensor_scalar(
                        es[:].bitcast(I16), ps[:], EXP_A, EXP_B,
                        mybir.AluOpType.mult, mybir.AluOpType.add,
                    )
                else:
                    nc.scalar.activation(
                        es[:], ps[:], mybir.ActivationFunctionType.Exp,
                        bias=0.0, scale=SCALE,
                    )
                return es

            def emit_av(pavs, es, jt, hp):
                for hh in range(2):
                    h = hp * 2 + hh
                    nc.tensor.matmul(
                        pavs[hh][:DH + 1, :],
                        VP[:, jt, h * (DH + 1):(h + 1) * (DH + 1)],
                        es[:, hh, :],
                        start=(jt == 0), stop=(jt == NJT - 1),
                    )

            def oproj_units(ib):
                units = []
                for s in range(ib * (IBLK // P), (ib + 1) * (IBLK // P)):
                    for nb in range(DIM // IBLK):
                        def go(s=s, nb=nb):
                            po = psB.tile([P, IBLK], F32, tag="po", bufs=2,
                                          name=f"po_{s}_{nb}")
                            for e in range(NEMBC):
                                nc.tensor.matmul(
                                    po[:],
                                    outT[:, e, s * P:(s + 1) * P],
                                    wo_sb[:, e, nb * IBLK:(nb + 1) * IBLK],
                                    start=(e == 0), stop=(e == NEMBC - 1),
                                )
                            oc = stage_pool.tile([P, IBLK], F32, tag="oc",
                                                 bufs=3, name=f"oc_{s}_{nb}")
                            if s % 2 == 0:
                                nc.scalar.activation(
                                    oc[:], po[:],
                                    mybir.ActivationFunctionType.Copy,
                                    bias=0.0, scale=1.0,
                                )
                            else:
                                nc.vector.tensor_copy(oc[:], po[:])
                            nc.sync.dma_start(
                                out[s * P:(s + 1) * P, nb * IBLK:(nb + 1) * IBLK],
                                oc[:],
                            )
                        units.append(go)
                return units

            pending = []
            div2 = []
            for ib in range(NIBLK):
                i0 = ib * IBLK
                for hp in range(2):
                    pavs = [
                        psB.tile([P, IBLK], F32, tag="pav", bufs=2,
                                 name=f"pav_{hp}_{hh}_{ib}")
                        for hh in range(2)
                    ]
                    prev = None
                    n_fill = len(pending)
                    for jt in range(NJT):
                        es = emit_spair(ib, jt, hp)
                        if div2 and jt < 2:
                            div2.pop(0)()
                        if prev is not None:
                            emit_av(pavs, prev, jt - 1, hp)
                        if n_fill > 0 and jt % 2 == 1:
                            pending.pop(0)()
                            n_fill -= 1
                        prev = es
                    emit_av(pavs, prev, NJT - 1, hp)

                    # AV psum -> sbuf bf16 (frees psum slots fast); den row
                    # psum -> sbuf via DMA (zero engine cost); divide tail
                    # deferred into the next pass as PE filler
                    for hh in range(2):
                        h = hp * 2 + hh
                        pavc = stage_pool.tile([DH, IBLK], BF16, tag="pavc",
                                               bufs=2, name=f"pavc_{h}_{ib}")
                        nc.vector.tensor_copy(pavc[:], pavs[hh][:DH, :])
                        den_row = stage_pool.tile([1, IBLK], F32R, tag="den",
                                                  bufs=2, name=f"den_{h}_{ib}")
                        nc.vector.tensor_copy(
                            den_row[:], pavs[hh][DH:DH + 1, :].bitcast(F32R))

                        def div_tail(h=h, i0=i0, ib=ib, pavc=pavc,
                                     den_row=den_row):
                            recb_ps = psB.tile([P, IBLK], F32, tag="po", bufs=2,
                                               name=f"recb_{h}_{ib}")
                            nc.tensor.matmul(
                                recb_ps[:DH, :], ones_t[:], den_row[:],
                                start=True, stop=True,
                            )
                            recb_sb = stage_pool.tile([DH, IBLK], F32,
                                                      tag="recb", bufs=2)
                            nc.vector.reciprocal_approx_fast(
                                recb_sb[:], recb_ps[:DH, :])
                            e_c, e_lo = divmod(h * DH, P)
                            nc.vector.tensor_tensor(
                                outT[e_lo:e_lo + DH, e_c, i0:i0 + IBLK],
                                pavc[:], recb_sb[:],
                                mybir.AluOpType.mult,
                            )
                        div2.append(div_tail)

                pending.extend(oproj_units(ib))

            for go in div2:
                go()
            for go in pending:
                go()

            es_ctx.__exit__(None, None, None)
            psB_ctx.__exit__(None, None, None)

    nc.compile()
    return nc


def shard_inputs(inputs):
    """Full inputs dict -> list of 8 per-core input dicts."""
    import ml_dtypes
    bf = ml_dtypes.bfloat16
    x = np.asarray(inputs["x"], dtype=np.float32).astype(bf)
    maps = []
    for core in range(8):
        bi, hg = divmod(core, 4)
        sl = slice(hg * EMB_C, (hg + 1) * EMB_C)
        maps.append({
            "xbf": np.ascontiguousarray(x[bi]),
            "wq": np.ascontiguousarray(np.asarray(inputs["wq"], np.float32)[:, sl]).astype(bf),
            "wk": np.ascontiguousarray(np.asarray(inputs["wk"], np.float32)[:, sl]).astype(bf),
            "wv": np.ascontiguousarray(np.asarray(inputs["wv"], np.float32)[:, sl]).astype(bf),
            "bq": np.ascontiguousarray(np.asarray(inputs["bq"], np.float32)[sl]),
            "bk": np.ascontiguousarray(np.asarray(inputs["bk"], np.float32)[sl]),
            "wo": np.ascontiguousarray(np.asarray(inputs["wo"], np.float32)[sl, :]).astype(bf),
            "ones64": np.ones((1, DH), np.float32),
        })
    return maps


def gather_outputs(results, bv, wo, bo):
    out = np.zeros((2, SEQ, DIM), np.float32)
    for core in range(8):
        bi = core // 4
        out[bi] += results[core]["out"]
    bo_eff = (np.asarray(bo, np.float64)
              + np.asarray(bv, np.float64) @ np.asarray(wo, np.float64))
    out += bo_eff.astype(np.float32)
    return out


_NC_CACHE = {}


def _get_nc(row_pack=True):
    if row_pack not in _NC_CACHE:
        _NC_CACHE[row_pack] = build_kernel(row_pack=row_pack)
    return _NC_CACHE[row_pack]


def run_sharded(inputs, trace=False, row_pack=True):
    """Returns (full_output [2,2048,1024] fp32, BassKernelResults)."""
    from concourse import bass_utils
    nc = _get_nc(row_pack)
    maps = shard_inputs(inputs)
    res = bass_utils.run_bass_kernel_spmd(
        nc, maps, core_ids=list(range(8)), trace=trace,
    )
    out = gather_outputs(res.results, np.asarray(inputs["bv"]),
                         np.asarray(inputs["wo"]), np.asarray(inputs["bo"]))
    return out, res


def kernel(**inputs):
    out, _ = run_sharded(inputs)
    return out


# revision 28
# speedup vs baseline: 1.1308x; 1.1308x over previous
"""Bass/Tile kernel for nn_Attention_81690277970645 on TRN2 (v2).

Sharding: 8 cores = 2 batches x 4 head-groups (4 heads of d=64 each).
Per core (batch bi, head-group hg):
  inputs:  xbf [2048, 1024] bf16, wq/wk/wv slices [1024, 256] bf16,
           bq/bk [256] f32, wo slice [256, 1024] bf16, ones64 [1,64] bf16
  output:  partial out [2048, 1024] f32 (host sums the 4 head-group
           partials per batch and adds bo_eff = bo + bv @ wo)

v2 changes vs v1 (271us):
  1. x^T via DMA-transpose (XBAR) in bf16 -- frees PE transposes and the
     psum->sbuf copy engines entirely.
  2. All matmuls bf16 (weights host-cast); V bias folded into host bo_eff
     (softmax rows sum to 1 => attn @ (1 bv^T) Wo = bv Wo = const row).
  3. Softmax exp split across ACT and DVE per-jt: ACT runs exact Exp
     activation; DVE computes Schraudolph fast-exp in ONE tensor_scalar:
     bitcast_bf16(int16(S * SCALE*128/ln2 + (127*128-7.25))) -- measured
     end-to-end rel err ~5e-3 (gate 2e-2).
  4. Output projection psum -> DRAM directly via DMA (no engine copy).
  5. GPSIMD unusable (cannot access PSUM on TRN2).

Known HW pitfalls: DVE reciprocal() on a 1-partition row costs ~3.3us;
reciprocal_approx_fast returns garbage at base partition != 0; gpsimd
cannot touch PSUM; dma_start_transpose needs contiguous SBUF dest.
"""
import sys
import numpy as np

if '/opt/trn_rl_repo' not in sys.path:
    sys.path.insert(0, '/opt/trn_rl_repo')

import concourse.mybir as mybir
from concourse import bacc
from concourse.tile import TileContext

F32 = mybir.dt.float32
F32R = mybir.dt.float32r
BF16 = mybir.dt.bfloat16
I16 = mybir.dt.int16

SEQ = 2048
DIM = 1024
EMB_C = 256          # per-core emb columns (4 heads x 64)
NH = 4               # heads per core
DH = 64
SCALE = DH ** -0.5
P = 128
NSEQT = SEQ // P     # 16 seq tiles
NDIMC = DIM // P     # 8 dim chunks
NEMBC = EMB_C // P   # 2 emb chunks
IBLK = 512
NIBLK = SEQ // IBLK  # 4 i-blocks
NJT = SEQ // P       # 16 j tiles

LN2 = 0.6931471805599453
EXP_A = SCALE * 128.0 / LN2      # schraudolph scale (SCALE folded in)
EXP_B = 127.0 * 128.0 - 7.25     # schraudolph bias

# which jt's exp runs on DVE (rest on ACT); ~44% DVE
DVE_JT = frozenset((1, 3, 5, 7, 9, 11, 13))


def build_kernel(row_pack=False):
    nc = bacc.Bacc("TRN2", target_bir_lowering=False, debug=False, num_devices=8)

    x = nc.dram_tensor("xbf", [SEQ, DIM], BF16, kind="ExternalInput")
    wq = nc.dram_tensor("wq", [DIM, EMB_C], BF16, kind="ExternalInput")
    wk = nc.dram_tensor("wk", [DIM, EMB_C], BF16, kind="ExternalInput")
    wv = nc.dram_tensor("wv", [DIM, EMB_C], BF16, kind="ExternalInput")
    bq = nc.dram_tensor("bq", [EMB_C], F32, kind="ExternalInput")
    bk = nc.dram_tensor("bk", [EMB_C], F32, kind="ExternalInput")
    wo = nc.dram_tensor("wo", [EMB_C, DIM], BF16, kind="ExternalInput")
    ones_d = nc.dram_tensor("ones64", [1, DH], F32, kind="ExternalInput")
    out = nc.dram_tensor("out", [SEQ, DIM], F32, kind="ExternalOutput")

    with TileContext(nc) as tc:
        with (
            tc.tile_pool(name="xt", bufs=1) as xt_pool,
            tc.tile_pool(name="w", bufs=1) as w_pool,
            tc.tile_pool(name="big", bufs=1) as big_pool,
            tc.tile_pool(name="stage", bufs=3) as stage_pool,
        ):
            # ---- stage A: weights + x^T via DMA transpose ----
            # xT lives in the FIRST pool: the XBAR dma-transpose writes
            # garbage into even partitions when the SBUF dest is not
            # 512B-aligned (a 256B tile allocated before these broke it).
            xT = []
            for b in range(NIBLK):
                t = xt_pool.tile([P, NDIMC, IBLK], BF16, name=f"xT_{b}")
                nc.sync.dma_start_transpose(t[:], x[b * IBLK:(b + 1) * IBLK, :])
                xT.append(t)

            wk_sb = w_pool.tile([P, NDIMC, EMB_C], BF16)
            nc.sync.dma_start(wk_sb[:], wk.rearrange("(c p) e -> p c e", p=P))
            bk_sb = w_pool.tile([P, NEMBC], F32)
            nc.sync.dma_start(bk_sb[:], bk.rearrange("(c p) -> p c", p=P))
            wv_sb = w_pool.tile([P, NDIMC, EMB_C], BF16)
            nc.sync.dma_start(wv_sb[:], wv.rearrange("(c p) e -> p c e", p=P))
            wq_sb = w_pool.tile([P, NDIMC, EMB_C], BF16)
            nc.sync.dma_start(wq_sb[:], wq.rearrange("(c p) e -> p c e", p=P))
            bq_sb = w_pool.tile([P, NEMBC], F32)
            nc.sync.dma_start(bq_sb[:], bq.rearrange("(c p) -> p c", p=P))
            wo_sb = w_pool.tile([P, NEMBC, DIM], BF16)
            nc.sync.dma_start(wo_sb[:], wo.rearrange("(c p) n -> p c n", p=P))
            ones_t = w_pool.tile([1, DH], F32R)
            nc.sync.dma_start(ones_t[:], ones_d[:].bitcast(F32R))

            # ---- stage B: K^T, V, Q^T ----
            psA_ctx = tc.tile_pool(name="psA", bufs=1, space="PSUM")
            psA = psA_ctx.__enter__()

            QT = big_pool.tile([P, NEMBC, SEQ], BF16)
            KT = big_pool.tile([P, NEMBC, SEQ], BF16)
            VP = big_pool.tile([P, NSEQT, NH * (DH + 1)], BF16)
            for h in range(NH):
                nc.vector.memset(VP[:, :, h * (DH + 1) + DH], 1.0)

            def emit_proj(dst, wsb, bsb, e, ib):
                pq = psA.tile([P, IBLK], F32, tag="pq", bufs=2)
                for c in range(NDIMC):
                    nc.tensor.matmul(
                        pq[:],
                        wsb[:, c, e * P:(e + 1) * P],
                        xT[ib][:, c, :],
                        start=(c == 0), stop=(c == NDIMC - 1),
                    )
                nc.scalar.activation(
                    dst[:, e, ib * IBLK:(ib + 1) * IBLK], pq[:],
                    mybir.ActivationFunctionType.Identity,
                    bias=bsb[:, e:e + 1], scale=1.0,
                )

            def emit_v(s):
                ib, si = divmod(s, IBLK // P)
                pv = psA.tile([P, EMB_C], F32, tag="pv", bufs=2)
                for c in range(NDIMC):
                    nc.tensor.matmul(
                        pv[:],
                        xT[ib][:, c, si * P:(si + 1) * P],
                        wv_sb[:, c, :],
                        start=(c == 0), stop=(c == NDIMC - 1),
                    )
                nc.vector.tensor_copy(
                    VP[:, s, :].rearrange("p (h x) -> p h x", h=NH)[:, :, :DH],
                    pv[:].rearrange("p (h d) -> p h d", h=NH),
                )

            for ib in range(NIBLK):
                for e in range(NEMBC):
                    emit_proj(KT, wk_sb, bk_sb, e, ib)
            for s in range(NSEQT):
                emit_v(s)
            for ib in range(NIBLK):
                for e in range(NEMBC):
                    emit_proj(QT, wq_sb, bq_sb, e, ib)

            psA_ctx.__exit__(None, None, None)

            # ---- stages C+D: attention + output projection ----
            psB_ctx = tc.tile_pool(name="psB", bufs=1, space="PSUM")
            psB = psB_ctx.__enter__()
            es_ctx = tc.tile_pool(name="es", bufs=1)
            es_pool = es_ctx.__enter__()

            outT = big_pool.tile([P, NEMBC, SEQ], BF16)

            def emit_spair(ib, jt, hp):
                """S^T for head-pair hp at (ib, jt): 2 matmuls + one exp."""
                i0 = ib * IBLK
                ps = psB.tile([P, 2, IBLK], F32, tag="s0", bufs=2,
                              name=f"ps{hp}_{ib}_{jt}")
                for hh in range(2):
                    lo = hh * DH
                    nc.tensor.matmul(
                        ps[:, hh, :],
                        KT[lo:lo + DH, hp, jt * P:(jt + 1) * P],
                        QT[lo:lo + DH, hp, i0:i0 + IBLK],
                        start=True, stop=True,
                    )
                es = es_pool.tile([P, 2, IBLK], BF16, tag="es", bufs=4,
                                  name=f"es{hp}_{ib}_{jt}")
                if jt in DVE_JT:
                    nc.vector.tensor_scalar(
                        es[:].bitcast(I16), ps[:], EXP_A, EXP_B,
                        mybir.AluOpType.mult, mybir.AluOpType.add,
                    )
                else:
                    nc.scalar.activation(
                        es[:], ps[:], mybir.ActivationFunctionType.Exp,
                        bias=0.0, scale=SCALE,
                    )
                return es

            def emit_av(pavs, es, jt, hp):
                for hh in range(2):
                    h = hp * 2 + hh
                    nc.tensor.matmul(
                        pavs[hh][:DH + 1, :],
                        VP[:, jt, h * (DH + 1):(h + 1) * (DH + 1)],
                        es[:, hh, :],
                        start=(jt == 0), stop=(jt == NJT - 1),
                    )

            def oproj_units(ib):
                units = []
                for s in range(ib * (IBLK // P), (ib + 1) * (IBLK // P)):
                    for nb in range(DIM // IBLK):
                        def go(s=s, nb=nb):
                            po = psB.tile([P, IBLK], F32, tag="po", bufs=2,
                                          name=f"po_{s}_{nb}")
                            for e in range(NEMBC):
                                nc.tensor.matmul(
                                    po[:],
                                    outT[:, e, s * P:(s + 1) * P],
                                    wo_sb[:, e, nb * IBLK:(nb + 1) * IBLK],
                                    start=(e == 0), stop=(e == NEMBC - 1),
                                )
                            oc = stage_pool.tile([P, IBLK], F32, tag="oc",
                                                 bufs=3, name=f"oc_{s}_{nb}")
                            if s % 2 == 0:
                                nc.scalar.activation(
                                    oc[:], po[:],
                                    mybir.ActivationFunctionType.Copy,
                                    bias=0.0, scale=1.0,
                                )
                            else:
                                nc.vector.tensor_copy(oc[:], po[:])
                            nc.sync.dma_start(
                                out[s * P:(s + 1) * P, nb * IBLK:(nb + 1) * IBLK],
                                oc[:],
                            )
                        units.append(go)
                return units

            pending = []
            div2 = []
            for ib in range(NIBLK):
                i0 = ib * IBLK
                for hp in range(2):
                    pavs = [
                        psB.tile([P, IBLK], F32, tag="pav", bufs=2,
                                 name=f"pav_{hp}_{hh}_{ib}")
                        for hh in range(2)
                    ]
                    prev1 = prev2 = None
                    n_fill = len(pending)
                    for jt in range(NJT):
                        # fillers first so the in-order PE queue has ready
                        # work while S(jt) waits on the exp(jt-2) psum WAR.
                        # div tails MUST pop before the first oproj filler:
                        # oproj reads the outT those tails write.
                        if div2 and jt in (2, 3):
                            div2.pop(0)()
                        if n_fill > 0 and 4 <= jt < 12:
                            pending.pop(0)()
                            n_fill -= 1
                        es = emit_spair(ib, jt, hp)
                        if prev2 is not None:
                            emit_av(pavs, prev2, jt - 2, hp)
                        prev2, prev1 = prev1, es
                    emit_av(pavs, prev2, NJT - 2, hp)
                    emit_av(pavs, prev1, NJT - 1, hp)

                    # den row first (it gates the recb matmul on PE), then the
                    # AV psum -> sbuf bf16 copy; divide tail deferred into the
                    # next pass as PE filler
                    for hh in range(2):
                        h = hp * 2 + hh
                        den_row = stage_pool.tile([1, IBLK], F32R, tag="den",
                                                  bufs=2, name=f"den_{h}_{ib}")
                        nc.vector.tensor_copy(
                            den_row[:], pavs[hh][DH:DH + 1, :].bitcast(F32R))
                        pavc = stage_pool.tile([DH, IBLK], BF16, tag="pavc",
                                               bufs=2, name=f"pavc_{h}_{ib}")
                        nc.vector.tensor_copy(pavc[:], pavs[hh][:DH, :])

                        def div_tail(h=h, i0=i0, ib=ib, pavc=pavc,
                                     den_row=den_row):
                            recb_ps = psB.tile([P, IBLK], F32, tag="po", bufs=2,
                                               name=f"recb_{h}_{ib}")
                            nc.tensor.matmul(
                                recb_ps[:DH, :], ones_t[:], den_row[:],
                                start=True, stop=True,
                            )
                            recb_sb = stage_pool.tile([DH, IBLK], F32,
                                                      tag="recb", bufs=2)
                            nc.vector.reciprocal_approx_fast(
                                recb_sb[:], recb_ps[:DH, :])
                            e_c, e_lo = divmod(h * DH, P)
                            # NB gpsimd mult at dst base partition 64 produced
                            # wrong results (err 0.17) -- keep this on DVE
                            nc.vector.tensor_tensor(
                                outT[e_lo:e_lo + DH, e_c, i0:i0 + IBLK],
                                pavc[:], recb_sb[:],
                                mybir.AluOpType.mult,
                            )
                        div2.append(div_tail)

                pending.extend(oproj_units(ib))

            for go in div2:
                go()
            for go in pending:
                go()

            es_ctx.__exit__(None, None, None)
            psB_ctx.__exit__(None, None, None)

    nc.compile()
    return nc


def shard_inputs(inputs):
    """Full inputs dict -> list of 8 per-core input dicts."""
    import ml_dtypes
    bf = ml_dtypes.bfloat16
    x = np.asarray(inputs["x"], dtype=np.float32).astype(bf)
    maps = []
    for core in range(8):
        bi, hg = divmod(core, 4)
        sl = slice(hg * EMB_C, (hg + 1) * EMB_C)
        maps.append({
            "xbf": np.ascontiguousarray(x[bi]),
            "wq": np.ascontiguousarray(np.asarray(inputs["wq"], np.float32)[:, sl]).astype(bf),
            "wk": np.ascontiguousarray(np.asarray(inputs["wk"], np.float32)[:, sl]).astype(bf),
            "wv": np.ascontiguousarray(np.asarray(inputs["wv"], np.float32)[:, sl]).astype(bf),
            "bq": np.ascontiguousarray(np.asarray(inputs["bq"], np.float32)[sl]),
            "bk": np.ascontiguousarray(np.asarray(inputs["bk"], np.float32)[sl]),
            "wo": np.ascontiguousarray(np.asarray(inputs["wo"], np.float32)[sl, :]).astype(bf),
            "ones64": np.ones((1, DH), np.float32),
        })
    return maps


def gather_outputs(results, bv, wo, bo):
    out = np.zeros((2, SEQ, DIM), np.float32)
    for core in range(8):
        bi = core // 4
        out[bi] += results[core]["out"]
    bo_eff = (np.asarray(bo, np.float64)
              + np.asarray(bv, np.float64) @ np.asarray(wo, np.float64))
    out += bo_eff.astype(np.float32)
    return out


_NC_CACHE = {}


def _get_nc(row_pack=True):
    if row_pack not in _NC_CACHE:
        _NC_CACHE[row_pack] = build_kernel(row_pack=row_pack)
    return _NC_CACHE[row_pack]


def run_sharded(inputs, trace=False, row_pack=True):
    """Returns (full_output [2,2048,1024] fp32, BassKernelResults)."""
    from concourse import bass_utils
    nc = _get_nc(row_pack)
    maps = shard_inputs(inputs)
    res = bass_utils.run_bass_kernel_spmd(
        nc, maps, core_ids=list(range(8)), trace=trace,
    )
    out = gather_outputs(res.results, np.asarray(inputs["bv"]),
                         np.asarray(inputs["wo"]), np.asarray(inputs["bo"]))
    return out, res


def kernel(**inputs):
    out, _ = run_sharded(inputs)
    return out
